# revision 2
# baseline (speedup 1.0000x reference)
"""Stochastic-computing bitstream AND-popcount kernel for 8 Trainium2 NeuronCores.

Reference computation:
    wbits[o,d,n] = (ranks[o,d,n] < round(clip(kernel[o,d],0,1)*128))   # fixed PRNG ranks
    out[b,o,n]   = (sum_d wbits[o,d,n] * inputs[b,d,n]) / 128

`ranks` depends only on jax.random.key(42) and the shapes, so it is a host
constant. The device work is 128 independent (64x1024)x(1024x512) matmuls
(one per bitstream position n), sharded over 8 cores by n (16 each).

Per-core device program (SPMD, no collectives):
  - x  : [128(dm), 16(nn), 8(dc), 64(b)]  fp8  (0.0/1.0)  ~1 MiB
  - w  : [128(dm), 16(nn), 8(dc), 512(o)] fp8  (0.0/1.0)  ~8.4 MiB
  - y  : [8(pair), 128(2x64 b), 512(o)]   fp16            ~1 MiB
  For each pair of n-positions: 8 contraction matmuls per n accumulate into
  one PSUM bank; the two n streams target array column halves (psum partitions
  0:64 / 64:128) so they execute concurrently on different PE column groups.
  ScalarE evicts PSUM with the 1/128 scale fused, casting to fp16.
"""

import os
import tempfile

import numpy as np
import ml_dtypes

B, D, N, O = 64, 1024, 128, 512
NCORES = 8
NPC = N // NCORES  # n positions per core
DC = D // 128      # contraction chunks of 128
PAIRS = NPC // 2

FP8_ONE = np.uint8(0x38)  # 1.0 in float8_e4m3

_RANKS_CACHE = os.path.join(tempfile.gettempdir(), "bitstream_ranks_dmndco_v1.npy")

_ranks_t = None    # [128(dm), 128(n), 8(dc), 512(o)] int8
_program = None    # compiled Bass program (module-level cache)


def _get_ranks_t():
    """Rank-of-each-position for the fixed key(42) permutations, pre-transposed
    to the device layout [dm, n, dc, o]. Constant across calls."""
    global _ranks_t
    if _ranks_t is not None:
        return _ranks_t
    if os.path.exists(_RANKS_CACHE):
        try:
            r = np.load(_RANKS_CACHE)
            if r.shape == (128, N, DC, O) and r.dtype == np.int8:
                _ranks_t = r
                return _ranks_t
        except Exception:
            pass
    import jax
    import jax.numpy as jnp

    with jax.default_device(jax.devices("cpu")[0]):
        u = jax.random.uniform(jax.random.key(42), (O, D, N))
        perm = np.asarray(jnp.argsort(u, axis=-1))
    # ranks = argsort(argsort(u)) == inverse permutation of argsort(u)
    ranks = np.empty((O, D, N), np.int8)
    np.put_along_axis(
        ranks, perm, np.broadcast_to(np.arange(N, dtype=np.int8), (O, D, N)), axis=-1
    )
    r = ranks.reshape(O, DC, 128, N).transpose(2, 3, 1, 0)  # [dm, n, dc, o]
    _ranks_t = np.ascontiguousarray(r)
    try:
        np.save(_RANKS_CACHE, _ranks_t)
    except Exception:
        pass
    return _ranks_t


def _build_program():
    global _program
    if _program is not None:
        return _program
    import concourse.bacc as bacc
    import concourse.mybir as mybir
    import concourse.tile as tile

    fp8 = mybir.dt.float8e4
    nc = bacc.Bacc(None, target_bir_lowering=False)

    x_d = nc.dram_tensor("x", [128, NPC, DC, B], fp8, kind="ExternalInput")
    w_d = nc.dram_tensor("w", [128, NPC, DC, O], fp8, kind="ExternalInput")
    y_d = nc.dram_tensor("y", [PAIRS, 128, O], mybir.dt.float16, kind="ExternalOutput")

    with tile.TileContext(nc) as tc:
        with (
            tc.tile_pool(name="xp", bufs=1) as xp,
            tc.tile_pool(name="wp", bufs=3) as wp,
            tc.tile_pool(name="pp", bufs=4, space="PSUM") as pp,
            tc.tile_pool(name="op", bufs=3) as op,
        ):
            x_sb = xp.tile([128, NPC, DC, B], fp8)
            nc.sync.dma_start(x_sb[:], x_d[:])
            for p in range(PAIRS):
                w_sb = wp.tile([128, 2, DC, O], fp8)
                nc.sync.dma_start(w_sb[:], w_d[:, 2 * p : 2 * p + 2])
                ps = pp.tile([128, O], mybir.dt.float32)
                # two n-streams on separate PE column halves, interleaved so
                # they run concurrently (tile_position auto-derived from the
                # psum slice base partition)
                for dc in range(DC):
                    nc.tensor.matmul(
                        ps[0:64, :],
                        x_sb[:, 2 * p, dc, :],
                        w_sb[:, 0, dc, :],
                        start=(dc == 0),
                        stop=(dc == DC - 1),
                    )
                    nc.tensor.matmul(
                        ps[64:128, :],
                        x_sb[:, 2 * p + 1, dc, :],
                        w_sb[:, 1, dc, :],
                        start=(dc == 0),
                        stop=(dc == DC - 1),
                    )
                o_sb = op.tile([128, O], mybir.dt.float16)
                nc.scalar.activation(
                    o_sb[:], ps[:], mybir.ActivationFunctionType.Copy, scale=1.0 / 128.0
                )
                nc.scalar.dma_start(y_d[p], o_sb[:])
    nc.compile()
    _program = nc
    return _program


def _prep_inputs(inputs, kernel):
    """Full inputs -> per-core in_maps (fp8 device layouts)."""
    ranks_t = _get_ranks_t()

    # weight bitstreams, directly in device layout [dm, n, dc, o]
    nb = np.round(np.clip(kernel, 0.0, 1.0) * np.float32(N)).astype(np.int16)  # (O, D)
    nb_m1 = (nb - 1).astype(np.int8).reshape(O, DC, 128).transpose(2, 1, 0)  # [dm,dc,o]
    wb = ranks_t <= nb_m1[:, None, :, :]  # bool [dm, n, dc, o]
    w8 = wb.view(np.uint8) * FP8_ONE

    # input bitstreams [dm, n, dc, b]
    x8 = (inputs.astype(np.uint8) * FP8_ONE).reshape(B, DC, 128, N).transpose(2, 3, 1, 0)

    in_maps = []
    for i in range(NCORES):
        sl = slice(NPC * i, NPC * (i + 1))
        in_maps.append(
            {
                "x": np.ascontiguousarray(x8[:, sl]).view(ml_dtypes.float8_e4m3),
                "w": np.ascontiguousarray(w8[:, sl]).view(ml_dtypes.float8_e4m3),
            }
        )
    return in_maps


def _assemble_output(results):
    out = np.empty((B, O, N), np.float32)
    for i, res in enumerate(results):
        y = np.asarray(res["y"])  # (PAIRS, 128, O) fp16
        out[:, :, NPC * i : NPC * (i + 1)] = (
            y.reshape(PAIRS, 2, B, O).transpose(2, 3, 0, 1).reshape(B, O, NPC)
        )
    return out


def run(inputs, kernel, trace=False):
    """Returns (output, BassKernelResults)."""
    from concourse.bass_utils import run_bass_kernel_spmd

    nc = _build_program()
    in_maps = _prep_inputs(np.asarray(inputs), np.asarray(kernel))
    bres = run_bass_kernel_spmd(nc, in_maps, list(range(NCORES)), trace=trace)
    return _assemble_output(bres.results), bres


def kernel(inputs, kernel):
    out, _ = run(inputs, kernel)
    return out


# revision 4
# speedup vs baseline: 1.1397x; 1.1397x over previous
"""Stochastic-computing bitstream AND-popcount kernel for 8 Trainium2 NeuronCores.

Reference computation:
    wbits[o,d,n] = (ranks[o,d,n] < round(clip(kernel[o,d],0,1)*128))   # fixed PRNG ranks
    out[b,o,n]   = (sum_d wbits[o,d,n] * inputs[b,d,n]) / 128

`ranks` depends only on jax.random.key(42) and the shapes, so it is a host
constant. The device work is 128 independent (64x1024)x(1024x512) matmuls
(one per bitstream position n), sharded over 8 cores by n (16 each).

Per-core device program (SPMD, no collectives):
  - x  : [128(dm), 16(nn), 8(dc), 64(b)]  fp8  (0.0/1.0)  ~1 MiB
  - w  : [128(dm), 16(nn), 8(dc), 512(o)] fp8  (0.0/1.0)  ~8.4 MiB
  - y  : [8(pair), 128(2x64 b), 512(o)]   fp16            ~1 MiB
  For each pair of n-positions: 8 contraction matmuls per n accumulate into
  one PSUM bank; the two n streams target array column halves (psum partitions
  0:64 / 64:128) so they execute concurrently on different PE column groups.
  ScalarE evicts PSUM with the 1/128 scale fused, casting to fp16.
"""

import os
import tempfile

import numpy as np
import ml_dtypes

B, D, N, O = 64, 1024, 128, 512
NCORES = 8
NPC = N // NCORES  # n positions per core
DC = D // 128      # contraction chunks of 128
PAIRS = NPC // 2

FP8_ONE = np.uint8(0x38)  # 1.0 in float8_e4m3

_RANKS_CACHE = os.path.join(tempfile.gettempdir(), "bitstream_ranks_dmndco_v1.npy")

_ranks_t = None    # [128(dm), 128(n), 8(dc), 512(o)] int8
_program = None    # compiled Bass program (module-level cache)


def _get_ranks_t():
    """Rank-of-each-position for the fixed key(42) permutations, pre-transposed
    to the device layout [dm, n, dc, o]. Constant across calls."""
    global _ranks_t
    if _ranks_t is not None:
        return _ranks_t
    if os.path.exists(_RANKS_CACHE):
        try:
            r = np.load(_RANKS_CACHE)
            if r.shape == (128, N, DC, O) and r.dtype == np.int8:
                _ranks_t = r
                return _ranks_t
        except Exception:
            pass
    import jax
    import jax.numpy as jnp

    with jax.default_device(jax.devices("cpu")[0]):
        u = jax.random.uniform(jax.random.key(42), (O, D, N))
        perm = np.asarray(jnp.argsort(u, axis=-1))
    # ranks = argsort(argsort(u)) == inverse permutation of argsort(u)
    ranks = np.empty((O, D, N), np.int8)
    np.put_along_axis(
        ranks, perm, np.broadcast_to(np.arange(N, dtype=np.int8), (O, D, N)), axis=-1
    )
    r = ranks.reshape(O, DC, 128, N).transpose(2, 3, 1, 0)  # [dm, n, dc, o]
    _ranks_t = np.ascontiguousarray(r)
    try:
        np.save(_RANKS_CACHE, _ranks_t)
    except Exception:
        pass
    return _ranks_t


def _build_program_tile():
    import concourse.bacc as bacc
    import concourse.mybir as mybir
    import concourse.tile as tile

    fp8 = mybir.dt.float8e4
    nc = bacc.Bacc(None, target_bir_lowering=False)

    x_d = nc.dram_tensor("x", [128, NPC, DC, B], fp8, kind="ExternalInput")
    w_d = nc.dram_tensor("w", [128, NPC, DC, O], fp8, kind="ExternalInput")
    y_d = nc.dram_tensor("y", [PAIRS, 128, O], mybir.dt.float16, kind="ExternalOutput")

    with tile.TileContext(nc) as tc:
        with (
            tc.tile_pool(name="xp", bufs=1) as xp,
            tc.tile_pool(name="wp", bufs=3) as wp,
            tc.tile_pool(name="pp", bufs=4, space="PSUM") as pp,
            tc.tile_pool(name="op", bufs=3) as op,
        ):
            x_sb = xp.tile([128, NPC, DC, B], fp8)
            nc.sync.dma_start(x_sb[:], x_d[:])
            for p in range(PAIRS):
                w_sb = wp.tile([128, 2, DC, O], fp8)
                nc.sync.dma_start(w_sb[:], w_d[:, 2 * p : 2 * p + 2])
                ps = pp.tile([128, O], mybir.dt.float32)
                # two n-streams on separate PE column halves, interleaved so
                # they run concurrently (tile_position auto-derived from the
                # psum slice base partition)
                for dc in range(DC):
                    nc.tensor.matmul(
                        ps[0:64, :],
                        x_sb[:, 2 * p, dc, :],
                        w_sb[:, 0, dc, :],
                        start=(dc == 0),
                        stop=(dc == DC - 1),
                    )
                    nc.tensor.matmul(
                        ps[64:128, :],
                        x_sb[:, 2 * p + 1, dc, :],
                        w_sb[:, 1, dc, :],
                        start=(dc == 0),
                        stop=(dc == DC - 1),
                    )
                o_sb = op.tile([128, O], mybir.dt.float16)
                nc.scalar.activation(
                    o_sb[:], ps[:], mybir.ActivationFunctionType.Copy, scale=1.0 / 128.0
                )
                nc.scalar.dma_start(y_d[p], o_sb[:])
    nc.compile()
    return nc


def _build_program_raw():
    """Hand-scheduled variant: no Tile framework barriers, explicit DMA pacing.

    Engine plan (per core):
      SYNC   triggers input DMAs (HWDGE): x, then w pair-chunks with two in
             flight so the stream stays saturated but the first pair lands
             early.
      TENSOR warms the PE (HAM) on dummy operands, then per pair runs the two
             n-streams interleaved on column halves of the array, accumulating
             over the 8 contraction chunks in one PSUM bank (4-bank rotation).
      VECTOR evicts PSUM -> fp16 SBUF with the 1/128 scale.
      SCALAR triggers output DMAs on its own HWDGE ring.
    """
    import concourse.bass as bass
    import concourse.mybir as mybir

    fp8 = mybir.dt.float8e4
    fp32 = mybir.dt.float32
    fp16 = mybir.dt.float16
    nc = bass.Bass(target_bir_lowering=False)

    x_d = nc.dram_tensor("x", [128, NPC, DC, B], fp8, kind="ExternalInput")
    w_d = nc.dram_tensor("w", [128, NPC, DC, O], fp8, kind="ExternalInput")
    y_d = nc.dram_tensor("y", [PAIRS, 128, O], fp16, kind="ExternalOutput")

    NBANK = 4
    WARMUP_MMS = 14

    with (
        nc.sbuf_tensor([128, NPC, DC, B], fp8) as x_sb,
        nc.sbuf_tensor([128, PAIRS, 2, DC, O], fp8) as w_sb,
        nc.sbuf_tensor([128, PAIRS, O], fp16) as o_sb,
        nc.sbuf_tensor([128, B], fp8) as dum_x,
        nc.sbuf_tensor([128, O], fp8) as dum_w,
        nc.psum_tensor([128, NBANK, O], fp32) as ps,
        nc.psum_tensor([128, O], fp32) as ps_scratch,
        nc.semaphore("x_sem") as x_sem,
        nc.semaphore("w_sem") as w_sem,
        nc.semaphore("mm_sem") as mm_sem,
        nc.semaphore("evac_sem") as evac_sem,
        nc.semaphore("y_sem") as y_sem,
        nc.Block() as block,
    ):

        @block.sync
        def _(sync: bass.BassEngine):
            sync.dma_start(out=x_sb[:], in_=x_d[:]).then_inc(x_sem, 16)
            sync.dma_start(out=w_sb[:, 0], in_=w_d[:, 0:2]).then_inc(w_sem, 16)
            sync.dma_start(out=w_sb[:, 1], in_=w_d[:, 2:4]).then_inc(w_sem, 16)
            for p in range(2, PAIRS):
                sync.wait_ge(w_sem, 16 * (p - 1))
                sync.dma_start(
                    out=w_sb[:, p], in_=w_d[:, 2 * p : 2 * p + 2]
                ).then_inc(w_sem, 16)
            sync.wait_ge(y_sem, 16 * PAIRS)

        @block.tensor
        def _(tensor: bass.BassEngine):
            # HAM warmup on garbage data while input DMAs stream
            for i in range(WARMUP_MMS):
                tensor.matmul(
                    ps_scratch[0:64, :],
                    dum_x[:, 0:64],
                    dum_w[:],
                    start=(i == 0),
                    stop=(i == WARMUP_MMS - 1),
                )
            tensor.wait_ge(x_sem, 16)
            for p in range(PAIRS):
                tensor.wait_ge(w_sem, 16 * (p + 1))
                if p >= NBANK:
                    tensor.wait_ge(evac_sem, p - NBANK + 1)
                bank = p % NBANK
                last = None
                for dc in range(DC):
                    tensor.matmul(
                        ps[0:64, bank, :],
                        x_sb[:, 2 * p, dc, :],
                        w_sb[:, p, 0, dc, :],
                        start=(dc == 0),
                        stop=(dc == DC - 1),
                    )
                    last = tensor.matmul(
                        ps[64:128, bank, :],
                        x_sb[:, 2 * p + 1, dc, :],
                        w_sb[:, p, 1, dc, :],
                        start=(dc == 0),
                        stop=(dc == DC - 1),
                    )
                last.then_inc(mm_sem, 1)

        @block.vector
        def _(vector: bass.BassEngine):
            for p in range(PAIRS):
                vector.wait_ge(mm_sem, p + 1)
                vector.tensor_scalar_mul(
                    o_sb[:, p, :], ps[:, p % NBANK, :], 1.0 / N
                ).then_inc(evac_sem, 1)

        @block.scalar
        def _(scalar: bass.BassEngine):
            for p in range(PAIRS):
                scalar.wait_ge(evac_sem, p + 1)
                scalar.dma_start(out=y_d[p], in_=o_sb[:, p, :]).then_inc(y_sem, 16)

    return nc


def _build_program():
    global _program
    if _program is not None:
        return _program
    if os.environ.get("BITSTREAM_KERNEL_VARIANT", "raw") == "tile":
        _program = _build_program_tile()
    else:
        _program = _build_program_raw()
    return _program


def _prep_inputs(inputs, kernel):
    """Full inputs -> per-core in_maps (fp8 device layouts)."""
    ranks_t = _get_ranks_t()

    # weight bitstreams, directly in device layout [dm, n, dc, o]
    nb = np.round(np.clip(kernel, 0.0, 1.0) * np.float32(N)).astype(np.int16)  # (O, D)
    nb_m1 = (nb - 1).astype(np.int8).reshape(O, DC, 128).transpose(2, 1, 0)  # [dm,dc,o]
    wb = ranks_t <= nb_m1[:, None, :, :]  # bool [dm, n, dc, o]
    w8 = wb.view(np.uint8) * FP8_ONE

    # input bitstreams [dm, n, dc, b]
    x8 = (inputs.astype(np.uint8) * FP8_ONE).reshape(B, DC, 128, N).transpose(2, 3, 1, 0)

    in_maps = []
    for i in range(NCORES):
        sl = slice(NPC * i, NPC * (i + 1))
        in_maps.append(
            {
                "x": np.ascontiguousarray(x8[:, sl]).view(ml_dtypes.float8_e4m3),
                "w": np.ascontiguousarray(w8[:, sl]).view(ml_dtypes.float8_e4m3),
            }
        )
    return in_maps


def _assemble_output(results):
    out = np.empty((B, O, N), np.float32)
    for i, res in enumerate(results):
        y = np.asarray(res["y"])  # (PAIRS, 128, O) fp16
        out[:, :, NPC * i : NPC * (i + 1)] = (
            y.reshape(PAIRS, 2, B, O).transpose(2, 3, 0, 1).reshape(B, O, NPC)
        )
    return out


def run(inputs, kernel, trace=False):
    """Returns (output, BassKernelResults)."""
    from concourse.bass_utils import run_bass_kernel_spmd

    nc = _build_program()
    in_maps = _prep_inputs(np.asarray(inputs), np.asarray(kernel))
    bres = run_bass_kernel_spmd(nc, in_maps, list(range(NCORES)), trace=trace)
    return _assemble_output(bres.results), bres


def kernel(inputs, kernel):
    out, _ = run(inputs, kernel)
    return out
